# revision 4
# baseline (speedup 1.0000x reference)
"""CharLSTMEmbedding Trainium2 kernel (fp8 DoubleRow recurrence).

Strategy (data-parallel over the flattened B*T=4096 word axis, 8 cores):
  - Words are globally sorted by char length (desc) and dealt round-robin to
    cores, so every core sees the same length profile (+-1 word per step).
    At char step t only the first N_t columns are computed;
    N_t = ceil(count(len > t) / 8) is a compile-time schedule.
  - The input-side gates G[id] (G = emb @ W_ih.T + bias, bf16) are gathered
    on the HOST per (word, step) and DMA'd in per step, chunk-major so every
    transfer is one contiguous segment per partition (the Sync engine's
    descriptor dispatch is the startup critical path).  They enter PSUM
    through identity matmuls (exact in bf16).
  - The recurrence matmuls run in fp8 (e4m3) DoubleRow mode: 2 matmuls of
    256-contraction replace 4 bf16 matmuls (measured 2.0x sustained).  The
    LDW-bound small-N tail steps use DoubleRowSwInterleave stationaries
    (contiguous weight reads; measured 1.7x over plain DoubleRow there) and
    issue the identity matmuls group-first (one LDWEIGHTS for all four).
    h is written as fp8 by the DVE (single rounding); W_hh is fp8 on host.
  - Activations stay fp32 (bf16 acts double the end-to-end error).
  - ACT (scalar engine) is the #1 bottleneck (1 elem/lane/cycle @1.2GHz,
    ~260ns/instr overhead): the i,f gates share one PSUM tile and are
    activated by a SINGLE fused sigmoid ACT over a 4D AP; step 0 (h=0)
    skips the f gate entirely and fuses sigmoid(i,o) the same way.
  - Elementwise work is split DVE/Pool: the critical c-chain (f*c + i*g),
    the fp8 h feedback and the masked copy_predicated stay on DVE; the
    i*g product and the output-region h muls run on the idle GpSimd.
  - Output h is maintained only over a small blend region per step (the
    ragged +-1 core-boundary window + the retiring slice), masked with
    copy_predicated, and streamed out by coalesced retirement DMAs.
  - PSUM packs two gate groups per bank-aligned [128,4,512] tile (column
    offsets 0/256), giving a 4-group-deep PE pipeline with 8 banks.
  - Step 0 needs no matmuls (h=0: gates come straight from the gin DMA);
    it runs in four column-quarters so step 1's matmuls start early.
  - Cross-step software pipelining: each step's words are split at N//2;
    fp8 h feedback lives in ping-pong tile pairs keyed to the NEXT step's
    halves, so next step's matmuls start stall-free.  gin is prefetched
    two steps ahead (the early-step transfers are ~5us each).

kernel(**inputs) takes the full unsharded inputs and returns [32,128,512] f32.
"""

import numpy as np
import ml_dtypes

B, T, L = 32, 128, 16
VOCAB, E, H = 256, 256, 512
NCORES = 8
BT = B * T
WPC = BT // NCORES  # 512 words per core
RCAP = 128          # max blend-region width supported by the program
SPLIT_MIN = 192     # steps with N > SPLIT_MIN run in two halves

LAST_RESULTS = None  # test harness can read exec_time_ns from here


def _gw(si):
    """gin block width for step si (step 0 drops the f gate)."""
    return 12 if si == 0 else 16


def _chunks(si, N):
    """Column chunks for step si (DMA granularity and, for step 0, the
    compute granularity)."""
    if si == 0:
        q = -(-N // 4)
        return [(a, min(a + q, N)) for a in range(0, N, q)]
    if N > SPLIT_MIN:
        return [(0, N // 2), (N // 2, N)]
    return [(0, N)]


def _build_program(steps, regions, tot_cols, mask_tot):
    """steps: list of (t, N, goff); goff is the absolute column offset of
    the step's gin block (step 0 is 12 blocks wide, others 16).
    regions: dict t -> (rlo, W, moff).
    Blend region [rlo, N): direct zone [rlo, N-W), masked zone [N-W, N)."""
    import concourse.bass as bass
    import concourse.tile as tile
    from concourse import bacc, mybir
    from contextlib import ExitStack

    f32 = mybir.dt.float32
    bf16 = mybir.dt.bfloat16
    fp8 = mybir.dt.float8e4
    u8 = mybir.dt.uint8
    AF = mybir.ActivationFunctionType
    DRSW = mybir.MatmulPerfMode.DoubleRowSwInterleave

    nc = bacc.Bacc("TRN2", target_bir_lowering=False, debug=False)

    gin_d = nc.dram_tensor("gin", [128, tot_cols], bf16, kind="ExternalInput")
    whsw_d = nc.dram_tensor("whsw", [128, 2 * 4096], fp8, kind="ExternalInput")
    idm_d = nc.dram_tensor("idm", [128, 128], bf16, kind="ExternalInput")
    if mask_tot > 0:
        mask_d = nc.dram_tensor("mask", [128, mask_tot], u8, kind="ExternalInput")
    hout_d = nc.dram_tensor("h_out", [128, 2048], f32, kind="ExternalOutput")
    hout_v = hout_d.rearrange("p (j n) -> p j n", j=4)

    with tile.TileContext(nc) as tc, ExitStack() as ctx:
        cpool = ctx.enter_context(tc.tile_pool(name="const", bufs=1))
        whsw_sb = cpool.tile([128, 2, 4096], fp8, name="whsw_sb", tag="whsw_sb")
        idm_sb = cpool.tile([128, 128], bf16, name="idm_sb", tag="idm_sb")
        if mask_tot > 0:
            mask_sb = cpool.tile([128, mask_tot], u8, name="mask_sb", tag="mask_sb")
        c_sb = cpool.tile([128, 4, 512], f32, name="c_sb", tag="c_sb")
        hout_sb = cpool.tile([128, 4, 512], f32, name="hout_sb", tag="hout_sb")
        h8A = [
            cpool.tile([128, 4, 256], fp8, name=f"h8A{j}", tag=f"h8A{j}")
            for j in range(2)
        ]
        h8B = [
            cpool.tile([128, 4, 256], fp8, name=f"h8B{j}", tag=f"h8B{j}")
            for j in range(2)
        ]

        gin_pool = ctx.enter_context(tc.tile_pool(name="gin", bufs=3))
        gate_pool = ctx.enter_context(tc.tile_pool(name="gps", bufs=2, space="PSUM"))
        act_pool = ctx.enter_context(tc.tile_pool(name="acts", bufs=1))
        tmp_pool = ctx.enter_context(tc.tile_pool(name="tmps", bufs=1))
        bl_pool = ctx.enter_context(tc.tile_pool(name="blend", bufs=2))

        # warm the ACT table before the main chain
        warm = cpool.tile([128, 8], f32, name="warm", tag="warm")
        nc.vector.memset(warm[:, :], 0.0)
        nc.scalar.activation(warm[:, :], warm[:, :], AF.Sigmoid)

        # pre-warm the PE's HAM clock gate: ~4us of dummy matmuls during the
        # DMA/step-0 startup window, so the real stream starts at 2.4 GHz
        # instead of running its first ~3.4us window at 1.2 GHz
        wsb = cpool.tile([128, 128], bf16, name="wsb", tag="wsb")
        nc.vector.memset(wsb[:, :], 0.0)
        wps = gate_pool.tile([128, 4, 512], f32, name="wps", tag="ps")
        for r in range(52):
            nc.tensor.matmul(
                wps[:, r % 4, :128], wsb[:, :], wsb[:, :],
                start=True, stop=True,
            )

        n_steps = len(steps)

        # gin tiles are chunk-major flat [128, gw*512]: chunk (a,b) of width w
        # occupies columns [gw*a, gw*b); inside it block m is
        # [gw*a + m*w, gw*a + (m+1)*w)
        def new_gin_tile(si):
            return gin_pool.tile(
                [128, 16 * 512], bf16, name=f"gin{si}", tag="gin"
            )

        def dma_gin_chunk(g_tile, si, a, b):
            gw = _gw(si)
            goff = steps[si][2]
            nc.sync.dma_start(
                g_tile[:, gw * a: gw * b],
                gin_d[:, goff + gw * a: goff + gw * b],
            )

        def dma_gin_full(g_tile, si):
            gw = _gw(si)
            goff = steps[si][2]
            N = steps[si][1]
            nc.sync.dma_start(
                g_tile[:, : gw * N], gin_d[:, goff: goff + gw * N]
            )

        def gin_m(g_tile, si, a, b, m):
            gw = _gw(si)
            w = b - a
            return g_tile[:, gw * a + m * w: gw * a + (m + 1) * w]

        def gin_blocks(g_tile, si, a, b, m0, m1):
            gw = _gw(si)
            w = b - a
            return g_tile[:, gw * a + m0 * w: gw * a + m1 * w
                          ].rearrange("p (m n) -> p m n", m=m1 - m0)

        # prefetch: step-0 gates first, identity, then step-1 first half so
        # step 1's matmuls unblock early, then the weights, then step 2.
        # dispatch order IS the startup critical path (~680ns of Sync per
        # DMA): step-1's first gate chunk (all its identity matmuls need)
        # goes right after step-0's first chunk and the identity matrix.
        g_tiles = {}
        g_tiles[0] = new_gin_tile(0)
        ch0 = _chunks(0, steps[0][1])
        dma_gin_chunk(g_tiles[0], 0, *ch0[0])
        nc.sync.dma_start(idm_sb[:, :], idm_d[:, :])
        if n_steps > 1:
            g_tiles[1] = new_gin_tile(1)
            ch1 = _chunks(1, steps[1][1])
            dma_gin_chunk(g_tiles[1], 1, *ch1[0])
        nc.sync.dma_start(
            whsw_sb[:, :, :], whsw_d.rearrange("p (k m) -> p k m", k=2)
        )
        for (a, b) in ch0[1:]:
            dma_gin_chunk(g_tiles[0], 0, a, b)
        if n_steps > 1:
            for (a, b) in ch1[1:]:
                dma_gin_chunk(g_tiles[1], 1, a, b)
        if n_steps > 2:
            g_tiles[2] = new_gin_tile(2)
            dma_gin_full(g_tiles[2], 2)
        if mask_tot > 0:
            nc.sync.dma_start(mask_sb[:, :], mask_d[:, :])

        last_retired = [steps[0][1]]
        for si, (t, N, goff) in enumerate(steps):
            first = si == 0
            last = si == n_steps - 1
            split = N > SPLIT_MIN
            Bs = N // 2 if split else N
            rA, rB = h8A[si % 2], h8B[si % 2]              # read set
            wA, wB = h8A[(si + 1) % 2], h8B[(si + 1) % 2]  # write set
            if not last:
                Nn = steps[si + 1][1]
                Bn = Nn // 2 if Nn > SPLIT_MIN else Nn
            else:
                Nn = Bn = 0
            chunks = _chunks(si, N)
            g_cur = g_tiles[si]

            # prefetch two steps ahead
            if si >= 1 and si + 2 < n_steps:
                g_tiles[si + 2] = new_gin_tile(si + 2)
                dma_gin_full(g_tiles[si + 2], si + 2)

            rlo, W, moff = regions[t]
            for hi, (s, e) in enumerate(chunks):
                n = e - s

                def mm_group(grp, ps):
                    """Matmuls for gate group grp into psum slice ps
                    ([128, 4, 256] view)."""

                    def dr_mms(m4):
                        m = grp * 4 + m4
                        for kk in range(2):
                            if e <= Bs:
                                rhs = rA[:, 2 * kk: 2 * kk + 2, s:e]
                            else:
                                rhs = rB[:, 2 * kk: 2 * kk + 2, s - Bs: e - Bs]
                            nc.tensor.matmul(
                                ps[:, m4, :n],
                                whsw_sb[:, kk, m * 256: (m + 1) * 256],
                                rhs, start=False, stop=(kk == 1),
                                perf_mode=DRSW,
                            )

                    if split:
                        # interleaved id/DR triples keep the PE at full rate
                        for m4 in range(4):
                            nc.tensor.matmul(
                                ps[:, m4, :n], idm_sb[:, :],
                                gin_m(g_cur, si, s, e, grp * 4 + m4),
                                start=True, stop=False,
                            )
                            dr_mms(m4)
                    else:
                        # LDW-bound tail: all id matmuls first (stationary
                        # reused), then the DRSW matmuls
                        for m4 in range(4):
                            nc.tensor.matmul(
                                ps[:, m4, :n], idm_sb[:, :],
                                gin_m(g_cur, si, s, e, grp * 4 + m4),
                                start=True, stop=False,
                            )
                        for m4 in range(4):
                            dr_mms(m4)

                if first:
                    # h == 0: gates come straight from the gin DMA; blocks
                    # are host-ordered [i(4), o(4), g(4)] (f is unused) so
                    # sigmoid(i,o) is ONE fused ACT.
                    iot = act_pool.tile(
                        [128, 8, 256], f32, name=f"io_{t}_{hi}",
                        tag=f"io{hi % 2}",
                    )
                    nc.scalar.activation(
                        iot[:, :, :n], gin_blocks(g_cur, si, s, e, 0, 8),
                        AF.Sigmoid,
                    )
                    gt = act_pool.tile(
                        [128, 4, 256], f32, name=f"g_{t}_{hi}",
                        tag=f"g{hi % 2}",
                    )
                    nc.scalar.activation(
                        gt[:, :, :n], gin_blocks(g_cur, si, s, e, 8, 12),
                        AF.Tanh,
                    )
                    it = iot[:, 0:4, :n]
                    ot = iot[:, 4:8, :n]
                    nc.vector.tensor_mul(
                        c_sb[:, :, s:e], it, gt[:, :, :n]
                    )
                else:
                    # pair tile A: i (cols 0:256) and f (cols 256:512);
                    # ONE fused sigmoid ACT over a 4D AP covers both.
                    psA = gate_pool.tile(
                        [128, 4, 512], f32, name=f"psA_{t}_{hi}", tag="ps"
                    )
                    mm_group(0, psA[:, :, 0:256])
                    mm_group(1, psA[:, :, 256:512])
                    ift = act_pool.tile(
                        [128, 4, 2, 256], f32, name=f"if_{t}_{hi}",
                        tag=f"if{hi % 2}",
                    )
                    psA4 = psA[:, :, :].rearrange("p m (g c) -> p m g c", g=2)
                    nc.scalar.activation(
                        ift[:, :, :, :n], psA4[:, :, :, :n], AF.Sigmoid
                    )
                    it = ift[:, :, 0, :n]
                    ft = ift[:, :, 1, :n]

                    psB = gate_pool.tile(
                        [128, 4, 512], f32, name=f"psB_{t}_{hi}", tag="ps"
                    )
                    mm_group(2, psB[:, :, 0:256])
                    gt = act_pool.tile(
                        [128, 4, 256], f32, name=f"g_{t}_{hi}",
                        tag=f"g{hi % 2}",
                    )
                    nc.scalar.activation(
                        gt[:, :, :n], psB[:, :, 0:0 + n], AF.Tanh
                    )
                    mm_group(3, psB[:, :, 256:512])

                    # c-chain stays on the DVE (critical path to tanh(c))
                    ig = tmp_pool.tile(
                        [128, 4, 256], f32,
                        name=f"ig{t}_{hi}", tag=f"ig{hi % 2}",
                    )
                    nc.vector.tensor_mul(ig[:, :, :n], it, gt[:, :, :n])
                    nc.vector.tensor_mul(
                        c_sb[:, :, s:e], ft, c_sb[:, :, s:e]
                    )
                    nc.vector.tensor_add(
                        c_sb[:, :, s:e], c_sb[:, :, s:e], ig[:, :, :n]
                    )

                # o before tanh(c) in the ACT FIFO: o's psum is ready early
                # (its matmuls were issued above) so ACT isn't head-blocked
                # waiting on the DVE c-chain.
                if not first:
                    ot = act_pool.tile(
                        [128, 4, 256], f32, name=f"o_{t}_{hi}",
                        tag=f"o{hi % 2}",
                    )
                    nc.scalar.activation(
                        ot[:, :, :n], psB[:, :, 256:256 + n], AF.Sigmoid
                    )
                    ot = ot[:, :, :n]

                # th is read by the (slow, off-critical) GpSimd output muls;
                # rotate 4 buffers so that reuse never gates the ACT stream.
                th = tmp_pool.tile(
                    [128, 4, 256], f32, name=f"th{t}_{hi}",
                    tag=f"th{si % 2}{hi % 2}",
                )
                nc.scalar.activation(th[:, :, :n], c_sb[:, :, s:e], AF.Tanh)

                # critical path: fp8 h tiles keyed to the NEXT step's halves
                if not last:
                    lo, hi_ = s, min(e, Bn)
                    if lo < hi_:
                        nc.vector.tensor_mul(
                            wA[:, :, lo:hi_],
                            ot[:, :, lo - s: hi_ - s], th[:, :, lo - s: hi_ - s],
                        )
                    lo, hi_ = max(s, Bn), min(e, Nn)
                    if lo < hi_:
                        nc.vector.tensor_mul(
                            wB[:, :, lo - Bn: hi_ - Bn],
                            ot[:, :, lo - s: hi_ - s], th[:, :, lo - s: hi_ - s],
                        )

                # off critical path: this chunk's part of the output
                # blend region [rlo, N): direct zone + masked window zone
                # (both muls on GpSimd; the masked merge needs the DVE)
                dlo, dhi = max(rlo, s), min(N - W, e)
                if dlo < dhi:
                    nc.gpsimd.tensor_mul(
                        hout_sb[:, :, dlo:dhi],
                        ot[:, :, dlo - s: dhi - s],
                        th[:, :, dlo - s: dhi - s],
                    )
                mlo = max(N - W, s)
                if W > 0 and mlo < e:
                    z0, z1 = mlo - (N - W), e - (N - W)
                    hw = bl_pool.tile(
                        [128, 4, RCAP], f32, name=f"hw{t}_{hi}", tag="hw"
                    )
                    nc.gpsimd.tensor_mul(
                        hw[:, :, z0:z1],
                        ot[:, :, mlo - s: e - s],
                        th[:, :, mlo - s: e - s],
                    )
                    mview = mask_sb[:, moff: moff + 4 * W].rearrange(
                        "p (j w) -> p j w", j=4
                    )
                    # mask=1 -> word still active on this core -> take new
                    nc.vector.copy_predicated(
                        hout_sb[:, :, mlo:e], mview[:, :, z0:z1],
                        hw[:, :, z0:z1],
                    )
                if e == N:
                    # retire [Nn, last_retired): final; stream out every
                    # few steps (each DMA costs ~115ns of epilogue drain)
                    if last or si % 4 == 3 or si == n_steps - 2:
                        nc.sync.dma_start(
                            hout_v[:, :, Nn:last_retired[0]],
                            hout_sb[:, :, Nn:last_retired[0]],
                        )
                        last_retired[0] = Nn

    nc.compile()
    return nc


def kernel(char_seq_padded, char_lengths, emb, W_ih, W_hh, b_ih, b_hh):
    global LAST_RESULTS
    from concourse.bass_utils import run_bass_kernel_spmd

    bf = ml_dtypes.bfloat16
    e4 = ml_dtypes.float8_e4m3

    char_seq_padded = np.asarray(char_seq_padded)
    ids_all = char_seq_padded.reshape(BT, L)
    lens = np.asarray(char_lengths).reshape(BT).astype(np.int64)
    emb = np.asarray(emb, dtype=np.float32)
    W_ih = np.asarray(W_ih, dtype=np.float32)
    W_hh = np.asarray(W_hh, dtype=np.float32)
    bias = np.asarray(b_ih, dtype=np.float32) + np.asarray(b_hh, dtype=np.float32)

    # ---- host precompute ----
    G_bf = (emb @ W_ih.T + bias).astype(bf)           # [VOCAB, 4H] bf16
    WhhT = np.ascontiguousarray(W_hh.T)               # [H, 4H]
    # DoubleRowSwInterleave stationaries: per (kpair, m) a flat [128, 256]
    # block, columns reversed, plane pairs interleaved:
    # whsw[p, kk, m, 2c+i] = WhhT[(2kk+i)*128+p, m*128 + 127-c]
    Wq = WhhT.astype(e4)
    R = Wq.reshape(2, 2, 128, 16, 128)[..., ::-1]     # [kk, i, p, m, c-rev]
    whsw_dev = np.ascontiguousarray(
        R.transpose(2, 0, 3, 4, 1).reshape(128, 2 * 16 * 256)
    )
    idm_dev = np.eye(128, dtype=bf)

    # ---- ragged schedule ----
    order = np.argsort(-lens, kind="stable")
    perms = [order[k::NCORES] for k in range(NCORES)]      # each [WPC], len-desc
    cnts = np.stack(
        [(lens[p][:, None] > np.arange(L)[None, :]).sum(0) for p in perms]
    )  # [NCORES, L]
    C = (lens[:, None] > np.arange(L)[None, :]).sum(0)     # [L] global counts

    raw = []      # (t, N)
    for t in range(L):
        if C[t] == 0:
            continue
        raw.append((t, int(-(-C[t] // NCORES))))

    steps = []       # (t, N, gin col offset)
    regions = {}     # t -> (rlo, W, mask_off)
    goff = 0
    moff = 0
    gin_core = [[] for _ in range(NCORES)]
    mask_core = [[] for _ in range(NCORES)]
    sel_all = [G_bf[ids_all[perms[k]]] for k in range(NCORES)]  # [WPC, L, 4H]
    # gate m-block order per step: step 0 drops f and goes [i, o, g] so
    # sigmoid(i,o) is one fused ACT; other steps keep [i, f, g, o]
    MB0 = [0, 1, 2, 3, 12, 13, 14, 15, 8, 9, 10, 11]
    for si, (t, N) in enumerate(raw):
        steps.append((t, N, goff))
        gw = 12 if si == 0 else 16
        goff += gw * N
        for k in range(NCORES):
            sel = sel_all[k][:N, t, :]                 # [N, 2048] bf16
            full = np.ascontiguousarray(
                sel.T.reshape(16, 128, N).transpose(1, 0, 2)
            )  # [128, 16, N]
            if si == 0:
                full = np.ascontiguousarray(full[:, MB0, :])  # [128, 12, N]
            blk = np.concatenate(
                [full[:, :, a:b].reshape(128, -1) for (a, b) in _chunks(si, N)],
                axis=1,
            )
            gin_core[k].append(np.ascontiguousarray(blk))
        W = int(N - cnts[:, t].min())
        if si + 1 < len(raw):
            tn, Nn = raw[si + 1]
            Wn = int(Nn - cnts[:, tn].min())
            rlo = max(min(N - W, Nn - Wn), 0)
        else:
            rlo = 0
        assert N - rlo <= RCAP, (t, N, rlo)
        regions[t] = (rlo, W, moff)
        if W > 0:
            moff += 4 * W
            for k in range(NCORES):
                m = (np.arange(N - W, N) < cnts[k, t]).astype(np.uint8)
                mask_core[k].append(np.tile(m, 4))
    tot_cols = goff
    mask_tot = moff

    nc = _build_program(steps, regions, tot_cols, mask_tot)

    in_maps = []
    for k in range(NCORES):
        m = {
            "gin": np.ascontiguousarray(np.concatenate(gin_core[k], axis=1)),
            "whsw": whsw_dev,
            "idm": idm_dev,
        }
        if mask_tot > 0:
            mrow = np.concatenate(mask_core[k])[None, :]
            m["mask"] = np.ascontiguousarray(np.repeat(mrow, 128, axis=0))
        in_maps.append(m)

    res = run_bass_kernel_spmd(nc, in_maps, list(range(NCORES)))
    LAST_RESULTS = res

    out = np.empty((BT, H), dtype=np.float32)
    for k in range(NCORES):
        hk = res.results[k]["h_out"]  # [128, 2048]
        out[perms[k]] = hk.reshape(128, 4, 512).transpose(2, 1, 0).reshape(WPC, H)
    return out.reshape(B, T, H)
